# revision 3
# baseline (speedup 1.0000x reference)
"""Trainium2 Bass kernel for block-causal attention (nn_Attn_63367947485870).

Sharding: tensor-parallel over heads x data-parallel over batch.
Core c (0..7): batch = c//4, heads = [4g..4g+3] where g = c%4.
Each core computes QKV projection for its 4 heads, RMSNorm+RoPE, block-causal
attention, and a partial out-projection [S, DM]; the host sums the 4 partials
per batch and adds the output bias.

On-device layouts (to keep the contraction dim on partitions):
  - x is passed pre-transposed per batch: xt [DM, S] (bf16)
  - q/k are computed transposed [feat, tok] in head-pair tiles [128, S]
  - v is computed in natural layout [tok, feat] (xT tiles as stationary lhsT)
  - scores are computed transposed [tk, tq]; softmax denominator comes from a
    ones-column appended to the PV lhsT (out row 64); no max-subtraction is
    needed because RMSNorm bounds |q.k|/8 <= 8.
"""

import sys

sys.path.insert(0, "/opt/trn_rl_repo")

import numpy as np

import concourse.bass as bass
import concourse.tile as tile
from concourse import mybir
from concourse import bass_utils
import bass_rust

B, S, DM = 2, 2048, 1024
H, HD = 16, 64
TPF = 128
NF = S // TPF  # 16 frames == 16 token tiles
EPS = 1e-6
ROPE_THETA = 10000.0

TC = 512          # token chunk (matmul moving dim)
NTC = S // TC     # 4
NTT = S // 128    # 16 token tiles
ND = DM // 128    # 8 contraction tiles

F32 = mybir.dt.float32
BF16 = mybir.dt.bfloat16
NPBF16 = mybir.dt.np(mybir.dt.bfloat16)

AF = mybir.ActivationFunctionType
OP = mybir.AluOpType


def _split_ctrl_waits(nc, max_waits=1):
    """Walrus in this container rejects instructions carrying more than one
    semaphore wait. Split extras into preceding single-wait NoOps on the same
    engine (queue order preserves the wait-before-execute semantics)."""
    for f in nc.m.functions:
        for blk in f.blocks:
            il = blk.instructions
            i = 0
            while i < len(il):
                inst = il[i]
                si = inst.sync_info
                waits = list(si.on_wait) if si and si.on_wait else []
                if len(waits) > max_waits:
                    keep = waits[:max_waits]
                    extra = waits[max_waits:]
                    pre = [
                        mybir.InstNoOp(
                            name=f"{inst.name}-wsplit{j}",
                            engine=inst.engine,
                            sync_info=bass_rust.SyncInfo(on_wait=[w], on_update=[]),
                        )
                        for j, w in enumerate(extra)
                    ]
                    si.on_wait = keep
                    for j, d in enumerate(pre):
                        il.insert(i + j, d)
                    i += len(pre)
                i += 1


def _build_program():
    nc = bass.Bass("TRN2", target_bir_lowering=False, debug=False)

    xt = nc.dram_tensor("xt", [DM, S], BF16, kind="ExternalInput")
    wqk = nc.dram_tensor("wqk", [DM, 512], BF16, kind="ExternalInput")
    wv = nc.dram_tensor("wv", [DM, 256], BF16, kind="ExternalInput")
    wo = nc.dram_tensor("wo", [256, DM], BF16, kind="ExternalInput")
    bqk = nc.dram_tensor("bqk", [128, 4], F32, kind="ExternalInput")
    bv = nc.dram_tensor("bv", [1, 256], F32, kind="ExternalInput")
    csq = nc.dram_tensor("csq", [128, S], BF16, kind="ExternalInput")
    snq = nc.dram_tensor("snq", [128, S], BF16, kind="ExternalInput")
    csk = nc.dram_tensor("csk", [128, S], BF16, kind="ExternalInput")
    snk = nc.dram_tensor("snk", [128, S], BF16, kind="ExternalInput")
    p2t = nc.dram_tensor("p2t", [128, 128], BF16, kind="ExternalInput")
    ones2 = nc.dram_tensor("ones2", [128, 2], BF16, kind="ExternalInput")
    bc2 = nc.dram_tensor("bc2", [2, 128], BF16, kind="ExternalInput")
    outp = nc.dram_tensor("outp", [S, DM], F32, kind="ExternalOutput")

    with tile.TileContext(nc) as tc:
        with (
            tc.tile_pool(name="const", bufs=1) as cpool,
            tc.tile_pool(name="big", bufs=1) as bpool,
            tc.tile_pool(name="tmp", bufs=3) as tpool,
            tc.tile_pool(name="attn", bufs=6) as apool,
            tc.tile_pool(name="outs", bufs=3) as opool,
            tc.tile_pool(name="psum", bufs=1, space="PSUM") as pspool,
        ):
            # ---- constant / input loads ----
            xts = []
            wqk_sb = []
            wv_sb = []
            for d in range(ND):
                t = cpool.tile([128, 512], BF16, tag=f"wqk{d}", name=f"wqk{d}")
                nc.sync.dma_start(out=t, in_=wqk[d * 128 : (d + 1) * 128, :])
                wqk_sb.append(t)
                t = bpool.tile([128, S], BF16, tag=f"xt{d}", name=f"xt{d}")
                nc.sync.dma_start(out=t, in_=xt[d * 128 : (d + 1) * 128, :])
                xts.append(t)
            for d in range(ND):
                t = cpool.tile([128, 256], BF16, tag=f"wv{d}", name=f"wv{d}")
                nc.sync.dma_start(out=t, in_=wv[d * 128 : (d + 1) * 128, :])
                wv_sb.append(t)
            wo_sb = []
            for p in range(2):
                t = cpool.tile([128, DM], BF16, tag=f"wo{p}", name=f"wo{p}")
                nc.sync.dma_start(out=t, in_=wo[p * 128 : (p + 1) * 128, :])
                wo_sb.append(t)
            bqk_sb = cpool.tile([128, 4], F32, tag="bqk", name="bqk")
            nc.sync.dma_start(out=bqk_sb, in_=bqk[:, :])
            bv_sb = cpool.tile([128, 256], F32, tag="bv", name="bv")
            bv_ap = bv[:, :]
            bv_bcast = bass.AP(
                tensor=bv_ap.tensor, offset=bv_ap.offset, ap=[[0, 128], bv_ap.ap[1]]
            )
            nc.gpsimd.dma_start(out=bv_sb, in_=bv_bcast)
            cs_sb = {}
            sn_sb = {}
            for qk, (c_t, s_t) in enumerate(((csq, snq), (csk, snk))):
                t = cpool.tile([128, S], BF16, tag=f"cs{qk}", name=f"cs{qk}")
                nc.sync.dma_start(out=t, in_=c_t[:, :])
                cs_sb[qk] = t
                t = cpool.tile([128, S], BF16, tag=f"sn{qk}", name=f"sn{qk}")
                nc.sync.dma_start(out=t, in_=s_t[:, :])
                sn_sb[qk] = t
            p2_sb = cpool.tile([128, 128], BF16, tag="p2", name="p2")
            nc.sync.dma_start(out=p2_sb, in_=p2t[:, :])
            ones2_sb = cpool.tile([128, 2], BF16, tag="ones2", name="ones2")
            nc.sync.dma_start(out=ones2_sb, in_=ones2[:, :])
            bc2_sb = cpool.tile([2, 128], BF16, tag="bc2", name="bc2")
            nc.sync.dma_start(out=bc2_sb, in_=bc2[:, :])
            ones64_sb = cpool.tile([128, 64], BF16, tag="ones64", name="ones64")
            nc.vector.memset(ones64_sb, 1.0)
            eps_sb = cpool.tile([2, 1], F32, tag="eps", name="eps")
            nc.vector.memset(eps_sb, EPS)

            # persistent activation tiles
            qb = [
                [bpool.tile([128, TC], BF16, tag=f"qb{p}_{c}", name=f"qb{p}_{c}") for c in range(NTC)]
                for p in range(2)
            ]
            kb = [
                [bpool.tile([128, TC], BF16, tag=f"kb{p}_{c}", name=f"kb{p}_{c}") for c in range(NTC)]
                for p in range(2)
            ]
            qh, kh = qb, kb  # rope output written in place
            vaug = [bpool.tile([128, 4, 65], BF16, tag=f"va{t_}", name=f"va{t_}") for t_ in range(NTT)]
            otp = [
                [bpool.tile([128, TC], BF16, tag=f"otp{p}_{c}", name=f"otp{p}_{c}") for c in range(NTC)]
                for p in range(2)
            ]

            for t_ in range(NTT):
                nc.vector.memset(vaug[t_][:, :, 64:65], 1.0)

            def emit_qkproj(ft):
                dst = qb[ft] if ft < 2 else kb[ft - 2]
                _ptag = {0: ("mm", 1), 1: ("rms", 1), 2: ("s", 2), 3: ("s", 2)}
                pss = {
                    c: pspool.tile(
                        [128, TC], F32, tag=_ptag[c][0],
                        name=f"qkproj{c}", bufs=_ptag[c][1],
                    )
                    for c in range(NTC)
                }
                for d in range(ND):
                    for c in range(NTC):
                        nc.tensor.matmul(
                            pss[c],
                            wqk_sb[d][:, ft * 128 : (ft + 1) * 128],
                            xts[d][:, c * TC : (c + 1) * TC],
                            start=(d == 0),
                            stop=(d == ND - 1),
                        )
                for c in range(NTC):
                    nc.scalar.activation(
                        dst[c], pss[c], AF.Identity, bias=bqk_sb[:, ft : ft + 1]
                    )

            def emit_vnat(t_):
                ps = pspool.tile([128, 256], F32, tag="mm", name="vproj", bufs=1)
                for d in range(ND):
                    nc.tensor.matmul(
                        ps,
                        xts[d][:, t_ * 128 : (t_ + 1) * 128],
                        wv_sb[d],
                        start=(d == 0),
                        stop=(d == ND - 1),
                    )
                nc.vector.tensor_add(
                    vaug[t_][:, :, 0:64],
                    ps.rearrange("p (h e) -> p h e", h=4),
                    bv_sb.rearrange("p (h e) -> p h e", h=4),
                )

            def emit_ropenorm(pr, qk):
                src_t = (qb if qk == 0 else kb)[pr]
                dst = (qh if qk == 0 else kh)[pr]
                rb = tpool.tile([2, S], BF16, tag="rb", name="rb", bufs=2)
                for c in range(NTC):
                    sl = slice(c * TC, (c + 1) * TC)
                    q2 = tpool.tile([128, TC], BF16, tag="q2", name="q2")
                    nc.scalar.activation(q2, src_t[c], AF.Square)
                    psr = pspool.tile([2, TC], F32, tag="rms", name="rms", bufs=1)
                    nc.tensor.matmul(psr, ones2_sb, q2, start=True, stop=True)
                    rr = tpool.tile([2, TC], F32, tag="rr", name="rr", bufs=2)
                    nc.scalar.activation(
                        rr, psr, AF.Sqrt, bias=eps_sb[:, 0:1], scale=1.0 / HD
                    )
                    with nc.allow_low_precision("bf16 rhs for rhat broadcast"):
                        nc.vector.reciprocal(rb[:, sl], rr)
                for c in range(NTC):
                    sl = slice(c * TC, (c + 1) * TC)
                    psp = pspool.tile([128, TC], F32, tag="mm", name="ropeP", bufs=1)
                    nc.tensor.matmul(psp, p2_sb, src_t[c], start=True, stop=True)
                    m1 = tpool.tile([128, TC], BF16, tag="qc", name="qc")
                    nc.vector.tensor_mul(m1, src_t[c], cs_sb[qk][:, sl])
                    m2 = tpool.tile([128, TC], BF16, tag="qs", name="qs")
                    nc.vector.tensor_mul(m2, psp, sn_sb[qk][:, sl])
                    m3 = tpool.tile([128, TC], BF16, tag="m3", name="m3")
                    nc.gpsimd.tensor_add(m3, m1, m2)
                    rhat_ps = pspool.tile([128, TC], F32, tag="s", name="rhat_ps", bufs=2)
                    nc.tensor.matmul(rhat_ps, bc2_sb, rb[:, sl], start=True, stop=True)
                    nc.vector.tensor_mul(dst[c], m3, rhat_ps)

            # projection + rope for pair 0 first, then pair 1; v just-in-time
            for pr in range(2):
                emit_qkproj(pr)        # q pair pr
                emit_qkproj(2 + pr)    # k pair pr
                emit_ropenorm(pr, 0)
                emit_ropenorm(pr, 1)

            # ---- phase 3+4: attention (qc outer), fused with out-projection ----
            for qc in range(NTC):
                ntk = 4 * qc + 4
                for t_ in range(4 * qc, 4 * qc + 4):
                    emit_vnat(t_)
                for pr in range(2):
                    oA = pspool.tile([65, TC], F32, tag="oA", name="oA", bufs=1)
                    oB = pspool.tile([65, TC], F32, tag="oB", name="oB", bufs=1)
                    att_tiles = {}
                    for tkp in range(0, ntk, 2):
                        pair = (tkp, tkp + 1)
                        voffs = [max(0, tk - 4 * qc) * 128 for tk in pair]
                        for half, (off0, off1) in enumerate(((0, 64), (64, 128))):
                            sps2 = pspool.tile(
                                [128, 2 * TC], F32, tag="s", name=f"s{half}", bufs=2
                            )
                            at2 = apool.tile(
                                [128, 2 * TC], BF16, tag=f"at{half}", name=f"at{half}", bufs=4
                            )
                            for j, tk in enumerate(pair):
                                voff = voffs[j]
                                ktile = kh[pr][tk // 4]
                                qtile = qh[pr][qc]
                                tko = (tk % 4) * 128
                                quirk = qc == 3 and tk == 0
                                qhi = 384 if quirk else TC
                                nc.tensor.matmul(
                                    sps2[:, j * TC + voff : j * TC + qhi],
                                    ktile[off0:off1, tko : tko + 128],
                                    qtile[off0:off1, voff:qhi],
                                    start=True,
                                    stop=True,
                                )
                            quirk0 = qc == 3 and tkp == 0
                            if voffs[0] == 0 and voffs[1] == 0 and not quirk0:
                                nc.scalar.activation(
                                    at2, sps2, AF.Exp, scale=HD**-0.5
                                )
                            else:
                                for j, tk in enumerate(pair):
                                    voff = voffs[j]
                                    qhi = 384 if (qc == 3 and tk == 0) else TC
                                    nc.scalar.activation(
                                        at2[:, j * TC + voff : j * TC + qhi],
                                        sps2[:, j * TC + voff : j * TC + qhi],
                                        AF.Exp,
                                        scale=HD**-0.5,
                                    )
                                    if qhi != TC:
                                        nc.vector.memset(at2[:, j * TC + qhi : (j + 1) * TC], 0.0)
                            att_tiles[(tkp, half)] = at2
                    for tk in range(ntk):
                        voff = max(0, tk - 4 * qc) * 128
                        j = tk % 2
                        for half, ps_out_tile in enumerate((oA, oB)):
                            h = 2 * pr + half
                            at2 = att_tiles[(tk - j, half)]
                            nc.tensor.matmul(
                                ps_out_tile[:, voff:TC],
                                vaug[tk][:, h, :],
                                at2[:, j * TC + voff : (j + 1) * TC],
                                start=(tk == 0),
                                stop=(tk == ntk - 1),
                            )
                    # normalize + evacuate into the pair chunk (odd half first:
                    # its extra SBUF-to-SBUF hop is on the critical path)
                    for half in (1, 0):
                        ps_out_tile = (oA, oB)[half]
                        dn = tpool.tile([65, TC], BF16, tag="dn", name="dn")
                        with nc.allow_low_precision("bf16 rhs for denom broadcast"):
                            nc.vector.reciprocal(dn[64:65, :], ps_out_tile[64:65, :])
                        ra_ps = pspool.tile([64, TC], F32, tag="s", name="ra_ps", bufs=2)
                        nc.tensor.matmul(
                            ra_ps, ones64_sb[64:65, :], dn[64:65, :], start=True, stop=True
                        )
                        ra = tpool.tile([64, TC], F32, tag="ra", name="ra")
                        nc.vector.tensor_copy(ra, ra_ps)
                        if half == 0:
                            nc.vector.tensor_mul(
                                otp[pr][qc][0:64, :], ps_out_tile[0:64, :], ra
                            )
                        else:
                            stg = tpool.tile([64, TC], BF16, tag="stg", name="stg")
                            nc.vector.tensor_mul(stg, ps_out_tile[0:64, :], ra)
                            nc.sync.dma_start(out=otp[pr][qc][64:128, :], in_=stg)
                # out-projection for this qc's 4 query tiles
                for qt in range(4 * qc, 4 * qc + 4):
                    tsl = slice(qt * 128, (qt + 1) * 128)
                    osl = slice((qt % 4) * 128, (qt % 4 + 1) * 128)
                    po0 = pspool.tile([128, 512], F32, tag="mm", name="po0", bufs=1)
                    po1 = pspool.tile([128, 512], F32, tag="rms", name="po1", bufs=1)
                    for p in range(2):
                        nc.tensor.matmul(
                            po0, otp[p][qc][:, osl], wo_sb[p][:, 0:512],
                            start=(p == 0), stop=(p == 1),
                        )
                        nc.tensor.matmul(
                            po1, otp[p][qc][:, osl], wo_sb[p][:, 512:1024],
                            start=(p == 0), stop=(p == 1),
                        )
                    ob = opool.tile([128, DM], F32, tag="ob", name="ob")
                    nc.vector.tensor_copy(ob[:, 0:512], po0)
                    nc.vector.tensor_copy(ob[:, 512:1024], po1)
                    nc.sync.dma_start(out=outp[tsl, :], in_=ob)

    _split_ctrl_waits(nc)
    return nc


_PROGRAM = None


def _get_program():
    global _PROGRAM
    if _PROGRAM is None:
        _PROGRAM = _build_program()
    return _PROGRAM


def _host_inputs(x, Wqkv, bqkv, q_scale, k_scale, Wout, bout):
    """Build the 8 per-core input maps."""
    inv_freq = 1.0 / (ROPE_THETA ** (np.arange(0, HD, 2, dtype=np.float64) / HD))
    pos = np.arange(S, dtype=np.float64)
    ang = pos[None, :] * inv_freq[:, None]  # [32, S]
    cos64 = np.concatenate([np.cos(ang), np.cos(ang)], axis=0)  # [64, S]
    sin64 = np.concatenate([np.sin(ang), np.sin(ang)], axis=0)
    cos128 = np.concatenate([cos64, cos64], axis=0)  # [128, S]
    sin128 = np.concatenate([sin64, sin64], axis=0)
    qs2 = np.tile(np.asarray(q_scale, np.float64), 2)[:, None]
    ks2 = np.tile(np.asarray(k_scale, np.float64), 2)[:, None]
    csq_np = (cos128 * qs2).astype(NPBF16)
    snq_np = (sin128 * qs2).astype(NPBF16)
    csk_np = (cos128 * ks2).astype(NPBF16)
    snk_np = (sin128 * ks2).astype(NPBF16)

    p2 = np.zeros((128, 128), dtype=np.float32)
    for base in (0, 64):
        for i in range(32):
            p2[base + i + 32, base + i] = -1.0  # lhsT[d, d'] = P[d', d]
            p2[base + i, base + i + 32] = 1.0
    p2 = p2.astype(NPBF16)

    ones2_np = np.zeros((128, 2), dtype=np.float32)
    ones2_np[0:64, 0] = 1.0
    ones2_np[64:128, 1] = 1.0
    ones2_np = ones2_np.astype(NPBF16)
    bc2_np = np.zeros((2, 128), dtype=np.float32)
    bc2_np[0, 0:64] = 1.0
    bc2_np[1, 64:128] = 1.0
    bc2_np = bc2_np.astype(NPBF16)

    Wq = Wqkv[0:DM]          # [1024, 1024] rows = head h dims
    Wk = Wqkv[DM : 2 * DM]
    Wv = Wqkv[2 * DM : 3 * DM]
    bq, bk, bv_full = bqkv[0:DM], bqkv[DM : 2 * DM], bqkv[2 * DM : 3 * DM]

    in_maps = []
    for c in range(8):
        b, g = divmod(c, 4)
        heads = [4 * g + i for i in range(4)]
        xtc = np.ascontiguousarray(x[b].T).astype(NPBF16)  # [DM, S]
        cols = []
        for pair in range(2):
            for h in heads[2 * pair : 2 * pair + 2]:
                cols.append(Wq[h * 64 : (h + 1) * 64])
        for pair in range(2):
            for h in heads[2 * pair : 2 * pair + 2]:
                cols.append(Wk[h * 64 : (h + 1) * 64])
        wqk_np = np.ascontiguousarray(np.concatenate(cols, axis=0).T).astype(NPBF16)
        wv_np = np.ascontiguousarray(
            np.concatenate([Wv[h * 64 : (h + 1) * 64] for h in heads], axis=0).T
        ).astype(NPBF16)
        wo_np = np.ascontiguousarray(
            np.concatenate([Wout[:, h * 64 : (h + 1) * 64].T for h in heads], axis=0)
        ).astype(NPBF16)
        bqk_np = np.zeros((128, 4), dtype=np.float32)
        for ft in range(4):
            bsrc = bq if ft < 2 else bk
            h0 = heads[2 * (ft % 2)]
            h1 = heads[2 * (ft % 2) + 1]
            bqk_np[0:64, ft] = bsrc[h0 * 64 : (h0 + 1) * 64]
            bqk_np[64:128, ft] = bsrc[h1 * 64 : (h1 + 1) * 64]
        bv_np = np.concatenate(
            [bv_full[h * 64 : (h + 1) * 64] for h in heads]
        ).astype(np.float32)[None, :]
        in_maps.append(
            {
                "xt": xtc,
                "wqk": wqk_np,
                "wv": wv_np,
                "wo": wo_np,
                "bqk": bqk_np,
                "bv": bv_np,
                "csq": csq_np,
                "snq": snq_np,
                "csk": csk_np,
                "snk": snk_np,
                "p2t": p2,
                "ones2": ones2_np,
                "bc2": bc2_np,
            }
        )
    return in_maps


def kernel(x, Wqkv, bqkv, q_scale, k_scale, Wout, bout, _trace=False, _results=None,
           _tmpdir=None):
    x = np.asarray(x, np.float32)
    Wqkv = np.asarray(Wqkv, np.float32)
    bqkv = np.asarray(bqkv, np.float32)
    Wout = np.asarray(Wout, np.float32)
    bout = np.asarray(bout, np.float32)

    nc = _get_program()
    in_maps = _host_inputs(x, Wqkv, bqkv, q_scale, k_scale, Wout, bout)
    res = bass_utils.run_bass_kernel_spmd(
        nc, in_maps, core_ids=list(range(8)), trace=_trace, tmpdir=_tmpdir
    )
    if _results is not None:
        _results.append(res)

    out = np.zeros((B, S, DM), dtype=np.float32)
    for c in range(8):
        b = c // 4
        out[b] += res.results[c]["outp"]
    out += bout[None, None, :]
    return out



# revision 14
# speedup vs baseline: 1.4482x; 1.4482x over previous
"""Trainium2 Bass kernel for block-causal attention (nn_Attn_63367947485870).

Sharding: tensor-parallel over heads x data-parallel over batch.
Core c (0..7): batch = c//4, heads = [4g..4g+3] where g = c%4.
Each core computes QKV projection for its 4 heads, RMSNorm+RoPE, block-causal
attention, and a partial out-projection [S, DM]; the host sums the 4 partials
per batch and adds the output bias.

On-device layouts (to keep the contraction dim on partitions):
  - x is passed pre-transposed per batch: xt [DM, S] (bf16)
  - q/k are computed transposed [feat, tok] as [128, S] head-pair tiles
  - v is computed in natural layout [tok, feat] (xT tiles as stationary lhsT)
  - scores are computed transposed [tk, tq]; softmax denominator comes from a
    ones-column appended to the PV lhsT (out row 64); no max-subtraction is
    needed because RMSNorm bounds |q.k|/8 <= 8.

Engine budget notes:
  - per-head score matmuls are K=64; the two heads of a pair are emitted
    adjacently at base partitions 0/64 so the PE runs them in separate
    row-groups concurrently.
  - 1/rms = exp(-0.5*ln(meansq)) on ScalarE (Rsqrt LUT is banned);
    1/denom = reciprocal_approx_fast on DVE; both broadcast across
    partitions via SBUF->SBUF DMA instead of PE matmuls.
"""

import sys

sys.path.insert(0, "/opt/trn_rl_repo")

import numpy as np

import concourse.bass as bass
import concourse.tile as tile
from concourse import mybir
from concourse import bass_utils
import bass_rust

B, S, DM = 2, 2048, 1024
H, HD = 16, 64
TPF = 128
NF = S // TPF  # 16 frames == 16 token tiles
EPS = 1e-6
ROPE_THETA = 10000.0

TC = 512          # token chunk (matmul moving dim)
NTC = S // TC     # 4
NTT = S // 128    # 16 token tiles
ND = DM // 128    # 8 contraction tiles

F32 = mybir.dt.float32
BF16 = mybir.dt.bfloat16
NPBF16 = mybir.dt.np(mybir.dt.bfloat16)

AF = mybir.ActivationFunctionType
OP = mybir.AluOpType


def _bcast_rows(ap, n):
    """AP that reads ap's single partition n times (partition-broadcast)."""
    return bass.AP(tensor=ap.tensor, offset=ap.offset, ap=[[0, n]] + list(ap.ap[1:]))


def _split_ctrl_waits(nc, max_waits=1):
    """Walrus in this container rejects instructions carrying more than one
    semaphore wait. Split extras into preceding single-wait NoOps on the same
    engine (queue order preserves the wait-before-execute semantics)."""
    for f in nc.m.functions:
        for blk in f.blocks:
            il = blk.instructions
            i = 0
            while i < len(il):
                inst = il[i]
                si = inst.sync_info
                waits = list(si.on_wait) if si and si.on_wait else []
                if len(waits) > max_waits:
                    keep = waits[:max_waits]
                    extra = waits[max_waits:]
                    pre = [
                        mybir.InstNoOp(
                            name=f"{inst.name}-wsplit{j}",
                            engine=inst.engine,
                            sync_info=bass_rust.SyncInfo(on_wait=[w], on_update=[]),
                        )
                        for j, w in enumerate(extra)
                    ]
                    si.on_wait = keep
                    for j, d in enumerate(pre):
                        il.insert(i + j, d)
                    i += len(pre)
                i += 1


def _build_program():
    nc = bass.Bass("TRN2", target_bir_lowering=False, debug=False)

    xt = nc.dram_tensor("xt", [DM, S], BF16, kind="ExternalInput")
    wqk = nc.dram_tensor("wqk", [DM, 512], BF16, kind="ExternalInput")
    wv = nc.dram_tensor("wv", [DM, 256], BF16, kind="ExternalInput")
    wo = nc.dram_tensor("wo", [256, DM], BF16, kind="ExternalInput")
    bqk = nc.dram_tensor("bqk", [128, 4], F32, kind="ExternalInput")
    bv = nc.dram_tensor("bv", [1, 256], F32, kind="ExternalInput")
    csq = nc.dram_tensor("csq", [128, S], BF16, kind="ExternalInput")
    snq = nc.dram_tensor("snq", [128, S], BF16, kind="ExternalInput")
    csk = nc.dram_tensor("csk", [128, S], BF16, kind="ExternalInput")
    snk = nc.dram_tensor("snk", [128, S], BF16, kind="ExternalInput")
    p2t = nc.dram_tensor("p2t", [128, 128], BF16, kind="ExternalInput")
    onesb = nc.dram_tensor("onesb", [128, 128], BF16, kind="ExternalInput")
    outp = nc.dram_tensor("outp", [S, DM], BF16, kind="ExternalOutput")

    with tile.TileContext(nc) as tc:
        with (
            tc.tile_pool(name="const", bufs=1) as cpool,
            tc.tile_pool(name="big", bufs=1) as bpool,
            tc.tile_pool(name="tmp", bufs=3) as tpool,
            tc.tile_pool(name="attn", bufs=6) as apool,
            tc.tile_pool(name="outs", bufs=3) as opool,
            tc.tile_pool(name="psum", bufs=1, space="PSUM") as pspool,
        ):
            # ---- constant / input loads ----
            xts = []
            wqk_sb = []
            wv_sb = []
            for d in range(ND):
                t = cpool.tile([128, 512], BF16, tag=f"wqk{d}", name=f"wqk{d}")
                nc.sync.dma_start(out=t, in_=wqk[d * 128 : (d + 1) * 128, :])
                wqk_sb.append(t)
                t = bpool.tile([128, S], BF16, tag=f"xt{d}", name=f"xt{d}")
                nc.sync.dma_start(out=t, in_=xt[d * 128 : (d + 1) * 128, :])
                xts.append(t)
            for d in range(ND):
                t = cpool.tile([128, 256], BF16, tag=f"wv{d}", name=f"wv{d}")
                nc.sync.dma_start(out=t, in_=wv[d * 128 : (d + 1) * 128, :])
                wv_sb.append(t)
            wo_sb = []
            for p in range(2):
                t = cpool.tile([128, DM], BF16, tag=f"wo{p}", name=f"wo{p}")
                nc.sync.dma_start(out=t, in_=wo[p * 128 : (p + 1) * 128, :])
                wo_sb.append(t)
            bqk_sb = cpool.tile([128, 4], F32, tag="bqk", name="bqk")
            nc.sync.dma_start(out=bqk_sb, in_=bqk[:, :])
            bv_sb = cpool.tile([128, 256], F32, tag="bv", name="bv")
            nc.gpsimd.dma_start(out=bv_sb, in_=_bcast_rows(bv[:, :], 128))
            cs_sb = {}
            sn_sb = {}
            for qk, (c_t, s_t) in enumerate(((csq, snq), (csk, snk))):
                t = cpool.tile([128, S], BF16, tag=f"cs{qk}", name=f"cs{qk}")
                nc.sync.dma_start(out=t, in_=c_t[:, :])
                cs_sb[qk] = t
                t = cpool.tile([128, S], BF16, tag=f"sn{qk}", name=f"sn{qk}")
                nc.sync.dma_start(out=t, in_=s_t[:, :])
                sn_sb[qk] = t
            p2_sb = cpool.tile([128, 128], BF16, tag="p2", name="p2")
            nc.sync.dma_start(out=p2_sb, in_=p2t[:, :])
            onesb_sb = cpool.tile([128, 128], BF16, tag="onesb", name="onesb")
            nc.sync.dma_start(out=onesb_sb, in_=onesb[:, :])
            eps_sb = cpool.tile([128, 1], F32, tag="eps", name="eps")
            nc.vector.memset(eps_sb, EPS)

            # persistent activation tiles: q/k head-pair tiles [feat, tok]
            qh = [bpool.tile([128, S], BF16, tag=f"qh{p}", name=f"qh{p}") for p in range(2)]
            kh = [bpool.tile([128, S], BF16, tag=f"kh{p}", name=f"kh{p}") for p in range(2)]
            # v in natural layout, augmented with 64 ones columns so the PV
            # matmul emits the softmax denominator replicated on rows 64..127
            vaug = [bpool.tile([128, 4, 128], BF16, tag=f"va{t_}", name=f"va{t_}") for t_ in range(NTT)]
            otp = [
                [bpool.tile([128, TC], BF16, tag=f"otp{p}_{c}", name=f"otp{p}_{c}") for c in range(NTC)]
                for p in range(2)
            ]

            for t_ in range(NTT):
                nc.vector.memset(vaug[t_][:, :, 64:128], 1.0)

            def emit_qkproj(ft):
                # ft: 0,1 = q pairs 0,1; 2,3 = k pairs 0,1
                dst = (qh if ft < 2 else kh)[ft % 2]
                for cp in range(2):
                    ps = pspool.tile(
                        [128, 2 * TC], F32, tag="s", name=f"qkp{ft}_{cp}", bufs=2
                    )
                    for d in range(ND):
                        for c2 in range(2):
                            nc.tensor.matmul(
                                ps[:, c2 * TC : (c2 + 1) * TC],
                                wqk_sb[d][:, ft * 128 : (ft + 1) * 128],
                                xts[d][:, (2 * cp + c2) * TC : (2 * cp + c2 + 1) * TC],
                                start=(d == 0),
                                stop=(d == ND - 1),
                            )
                    nc.scalar.activation(
                        dst[:, cp * 2 * TC : (cp + 1) * 2 * TC],
                        ps,
                        AF.Identity,
                        bias=bqk_sb[:, ft : ft + 1],
                    )

            def emit_vnat(t_):
                ps = pspool.tile([128, 256], F32, tag="aux", name="vproj", bufs=2)
                for d in range(ND):
                    nc.tensor.matmul(
                        ps,
                        xts[d][:, t_ * 128 : (t_ + 1) * 128],
                        wv_sb[d],
                        start=(d == 0),
                        stop=(d == ND - 1),
                    )
                nc.vector.tensor_add(
                    vaug[t_][:, :, 0:64],
                    ps.rearrange("p (h e) -> p h e", h=4),
                    bv_sb.rearrange("p (h e) -> p h e", h=4),
                )

            def emit_ropenorm(pr, qk):
                src_t = (qh if qk == 0 else kh)[pr]
                # -- 1/rms per token, replicated to all 64 feature rows of
                # each head via the block-ones lhsT: rhat = exp(-0.5*ln(ssq))
                q2 = tpool.tile([128, S], BF16, tag="q2", name="q2", bufs=2)
                nc.gpsimd.tensor_mul(q2, src_t, src_t)
                rhat = tpool.tile([128, S], BF16, tag="rhat", name="rhat", bufs=2)
                for c in range(NTC):
                    sl = slice(c * TC, (c + 1) * TC)
                    psr = pspool.tile([128, TC], F32, tag="aux", name="rms", bufs=2)
                    nc.tensor.matmul(psr, onesb_sb, q2[:, sl], start=True, stop=True)
                    lnr = tpool.tile([128, TC], F32, tag="lnr", name="lnr", bufs=3)
                    nc.scalar.activation(
                        lnr, psr, AF.Ln, bias=eps_sb[:, 0:1], scale=1.0 / HD
                    )
                    with nc.allow_low_precision("bf16 1/rms, matches baseline"):
                        nc.scalar.activation(rhat[:, sl], lnr, AF.Exp, scale=-0.5)
                # -- rope: dst = (t*cos + rot(t)*sin) * rhat, in place
                for c in range(NTC):
                    sl = slice(c * TC, (c + 1) * TC)
                    psp = pspool.tile([128, TC], F32, tag="aux", name="ropeP", bufs=2)
                    nc.tensor.matmul(psp, p2_sb, src_t[:, sl], start=True, stop=True)
                    m1 = tpool.tile([128, TC], BF16, tag="qc", name="qc")
                    nc.vector.tensor_mul(m1, src_t[:, sl], cs_sb[qk][:, sl])
                    m2 = tpool.tile([128, TC], BF16, tag="qs", name="qs")
                    nc.vector.tensor_mul(m2, psp, sn_sb[qk][:, sl])
                    m3 = tpool.tile([128, TC], BF16, tag="m3", name="m3")
                    nc.gpsimd.tensor_add(m3, m1, m2)
                    nc.vector.tensor_mul(src_t[:, sl], m3, rhat[:, sl])

            # ---- phase 1+2: projections + rope, pair 0 then pair 1 ----
            emit_qkproj(0)
            emit_qkproj(2)
            emit_ropenorm(0, 0)
            emit_ropenorm(0, 1)
            emit_qkproj(1)
            emit_qkproj(3)
            emit_ropenorm(1, 0)
            emit_ropenorm(1, 1)
            # v projection for all token tiles (fills PE while rope DVE runs)
            for t_ in range(NTT):
                emit_vnat(t_)

            # ---- phase 3+4: attention (qc outer), fused with out-projection ----
            for qc in range(NTC):
                ntk = 4 * qc + 4
                for pr in range(2):
                    oA = pspool.tile([128, TC], F32, tag="acc", name="oA", bufs=2)
                    oB = pspool.tile([128, TC], F32, tag="acc", name="oB", bufs=2)
                    att_tiles = {}
                    for tkp in range(0, ntk, 2):
                        pair = (tkp, tkp + 1)
                        voffs = [max(0, tk - 4 * qc) * 128 for tk in pair]
                        sps = {}
                        for half in range(2):
                            sps[half] = pspool.tile(
                                [128, 2 * TC], F32, tag="s", name=f"s{half}", bufs=2
                            )
                            att_tiles[(tkp, half)] = apool.tile(
                                [128, 2 * TC], BF16, tag=f"at{half}", name=f"at{half}", bufs=4
                            )
                        # emit the two halves' K=64 matmuls adjacently: they
                        # target row-groups 0/64 and run concurrently on the PE
                        for j, tk in enumerate(pair):
                            voff = voffs[j]
                            tko = tk * 128
                            quirk = qc == 3 and tk == 0
                            qhi = 384 if quirk else TC
                            for half, (off0, off1) in enumerate(((0, 64), (64, 128))):
                                nc.tensor.matmul(
                                    sps[half][:, j * TC + voff : j * TC + qhi],
                                    kh[pr][off0:off1, tko : tko + 128],
                                    qh[pr][off0:off1, qc * TC + voff : qc * TC + qhi],
                                    start=True,
                                    stop=True,
                                )
                        quirk0 = qc == 3 and tkp == 0
                        for half in range(2):
                            at2 = att_tiles[(tkp, half)]
                            if not quirk0:
                                nc.scalar.activation(
                                    at2, sps[half], AF.Exp, scale=HD**-0.5
                                )
                            else:
                                for j, tk in enumerate(pair):
                                    voff = voffs[j]
                                    qhi = 384 if (qc == 3 and tk == 0) else TC
                                    nc.scalar.activation(
                                        at2[:, j * TC + voff : j * TC + qhi],
                                        sps[half][:, j * TC + voff : j * TC + qhi],
                                        AF.Exp,
                                        scale=HD**-0.5,
                                    )
                                    if qhi != TC:
                                        nc.vector.memset(at2[:, j * TC + qhi : (j + 1) * TC], 0.0)
                    for tk in range(ntk):
                        voff = max(0, tk - 4 * qc) * 128
                        j = tk % 2
                        for half, ps_out_tile in enumerate((oA, oB)):
                            h = 2 * pr + half
                            at2 = att_tiles[(tk - j, half)]
                            nc.tensor.matmul(
                                ps_out_tile[:, voff:TC],
                                vaug[tk][:, h, :],
                                at2[:, j * TC + voff : (j + 1) * TC],
                                start=(tk == 0),
                                stop=(tk == ntk - 1),
                            )
                    # normalize + evacuate into the pair chunk (odd half first:
                    # its extra SBUF-to-SBUF hop is on the critical path).
                    # 1/denom = exp(-ln(d)), lane-parallel on rows 64..127
                    # (where the PV matmul replicated the denominator), then
                    # DMA'd down to rows 0..63 to meet the o dims.
                    for half in (1, 0):
                        ps_out_tile = (oA, oB)[half]
                        lnd = tpool.tile([128, TC], F32, tag="lnd", name="lnd", bufs=2)
                        nc.scalar.activation(
                            lnd[64:128, :], ps_out_tile[64:128, :], AF.Ln
                        )
                        raf = tpool.tile([128, TC], F32, tag="raf", name="raf", bufs=2)
                        nc.scalar.activation(
                            raf[64:128, :], lnd[64:128, :], AF.Exp, scale=-1.0
                        )
                        nc.sync.dma_start(out=raf[0:64, :], in_=raf[64:128, :])
                        if half == 0:
                            nc.vector.tensor_mul(
                                otp[pr][qc][0:64, :], ps_out_tile[0:64, :], raf[0:64, :]
                            )
                        else:
                            stg = tpool.tile([64, TC], BF16, tag="stg", name="stg")
                            nc.vector.tensor_mul(stg, ps_out_tile[0:64, :], raf[0:64, :])
                            nc.sync.dma_start(out=otp[pr][qc][64:128, :], in_=stg)
                # out-projection for this qc's 4 query tiles
                for qt in range(4 * qc, 4 * qc + 4):
                    tsl = slice(qt * 128, (qt + 1) * 128)
                    osl = slice((qt % 4) * 128, (qt % 4 + 1) * 128)
                    po0 = pspool.tile([128, 512], F32, tag="aux", name="po0", bufs=2)
                    po1 = pspool.tile([128, 512], F32, tag="aux", name="po1", bufs=2)
                    for p in range(2):
                        nc.tensor.matmul(
                            po0, otp[p][qc][:, osl], wo_sb[p][:, 0:512],
                            start=(p == 0), stop=(p == 1),
                        )
                        nc.tensor.matmul(
                            po1, otp[p][qc][:, osl], wo_sb[p][:, 512:1024],
                            start=(p == 0), stop=(p == 1),
                        )
                    ob = opool.tile([128, DM], BF16, tag="ob", name="ob")
                    with nc.allow_low_precision("bf16 partial out"):
                        nc.vector.tensor_copy(ob[:, 0:512], po0)
                        nc.vector.tensor_copy(ob[:, 512:1024], po1)
                    nc.sync.dma_start(out=outp[tsl, :], in_=ob)

    _split_ctrl_waits(nc)
    return nc


_PROGRAM = None


def _get_program():
    global _PROGRAM
    if _PROGRAM is None:
        _PROGRAM = _build_program()
    return _PROGRAM


def _host_inputs(x, Wqkv, bqkv, q_scale, k_scale, Wout, bout):
    """Build the 8 per-core input maps."""
    inv_freq = 1.0 / (ROPE_THETA ** (np.arange(0, HD, 2, dtype=np.float64) / HD))
    pos = np.arange(S, dtype=np.float64)
    ang = pos[None, :] * inv_freq[:, None]  # [32, S]
    cos64 = np.concatenate([np.cos(ang), np.cos(ang)], axis=0)  # [64, S]
    sin64 = np.concatenate([np.sin(ang), np.sin(ang)], axis=0)
    cos128 = np.concatenate([cos64, cos64], axis=0)  # [128, S]
    sin128 = np.concatenate([sin64, sin64], axis=0)
    qs2 = np.tile(np.asarray(q_scale, np.float64), 2)[:, None]
    ks2 = np.tile(np.asarray(k_scale, np.float64), 2)[:, None]
    csq_np = (cos128 * qs2).astype(NPBF16)
    snq_np = (sin128 * qs2).astype(NPBF16)
    csk_np = (cos128 * ks2).astype(NPBF16)
    snk_np = (sin128 * ks2).astype(NPBF16)

    p2 = np.zeros((128, 128), dtype=np.float32)
    for base in (0, 64):
        for i in range(32):
            p2[base + i + 32, base + i] = -1.0  # lhsT[d, d'] = P[d', d]
            p2[base + i, base + i + 32] = 1.0
    p2 = p2.astype(NPBF16)

    onesb_np = np.zeros((128, 128), dtype=np.float32)
    onesb_np[0:64, 0:64] = 1.0
    onesb_np[64:128, 64:128] = 1.0
    onesb_np = onesb_np.astype(NPBF16)

    Wq = Wqkv[0:DM]          # [1024, 1024] rows = head h dims
    Wk = Wqkv[DM : 2 * DM]
    Wv = Wqkv[2 * DM : 3 * DM]
    bq, bk, bv_full = bqkv[0:DM], bqkv[DM : 2 * DM], bqkv[2 * DM : 3 * DM]

    in_maps = []
    for c in range(8):
        b, g = divmod(c, 4)
        heads = [4 * g + i for i in range(4)]
        xtc = np.ascontiguousarray(x[b].T).astype(NPBF16)  # [DM, S]
        cols = []
        for pair in range(2):
            for h in heads[2 * pair : 2 * pair + 2]:
                cols.append(Wq[h * 64 : (h + 1) * 64])
        for pair in range(2):
            for h in heads[2 * pair : 2 * pair + 2]:
                cols.append(Wk[h * 64 : (h + 1) * 64])
        wqk_np = np.ascontiguousarray(np.concatenate(cols, axis=0).T).astype(NPBF16)
        wv_np = np.ascontiguousarray(
            np.concatenate([Wv[h * 64 : (h + 1) * 64] for h in heads], axis=0).T
        ).astype(NPBF16)
        wo_np = np.ascontiguousarray(
            np.concatenate([Wout[:, h * 64 : (h + 1) * 64].T for h in heads], axis=0)
        ).astype(NPBF16)
        bqk_np = np.zeros((128, 4), dtype=np.float32)
        for ft in range(4):
            bsrc = bq if ft < 2 else bk
            h0 = heads[2 * (ft % 2)]
            h1 = heads[2 * (ft % 2) + 1]
            bqk_np[0:64, ft] = bsrc[h0 * 64 : (h0 + 1) * 64]
            bqk_np[64:128, ft] = bsrc[h1 * 64 : (h1 + 1) * 64]
        bv_np = np.concatenate(
            [bv_full[h * 64 : (h + 1) * 64] for h in heads]
        ).astype(np.float32)[None, :]
        in_maps.append(
            {
                "xt": xtc,
                "wqk": wqk_np,
                "wv": wv_np,
                "wo": wo_np,
                "bqk": bqk_np,
                "bv": bv_np,
                "csq": csq_np,
                "snq": snq_np,
                "csk": csk_np,
                "snk": snk_np,
                "p2t": p2,
                "onesb": onesb_np,
            }
        )
    return in_maps


def kernel(x, Wqkv, bqkv, q_scale, k_scale, Wout, bout, _trace=False, _results=None,
           _tmpdir=None):
    x = np.asarray(x, np.float32)
    Wqkv = np.asarray(Wqkv, np.float32)
    bqkv = np.asarray(bqkv, np.float32)
    Wout = np.asarray(Wout, np.float32)
    bout = np.asarray(bout, np.float32)

    nc = _get_program()
    in_maps = _host_inputs(x, Wqkv, bqkv, q_scale, k_scale, Wout, bout)
    res = bass_utils.run_bass_kernel_spmd(
        nc, in_maps, core_ids=list(range(8)), trace=_trace, tmpdir=_tmpdir
    )
    if _results is not None:
        _results.append(res)

    out = np.zeros((B, S, DM), dtype=np.float32)
    for c in range(8):
        b = c // 4
        out[b] += np.asarray(res.results[c]["outp"], dtype=np.float32)
    out += bout[None, None, :]
    return out
